# revision 42
# baseline (speedup 1.0000x reference)
"""GCN layer (nn_GCNReg) on 8 Trainium2 NeuronCores.

Strategy (graph/data parallel, per sharding hint):
  - Nodes are partitioned across 8 cores by destination range (49 tiles of
    128 nodes per core).  Real edges are routed to the core owning their dst
    and sorted by dst.  Self-loop messages are NOT gathered: they are added
    per-batch from a plain load of a host-transposed copy of the core's own
    rows (an on-device XBAR transpose DMA serializes against the SWDGE
    gather stream and costs ~70us - avoid it).
  - Host preprocessing (free): deg/dinv from edge_index, xf = dinv * x in
    f16 (the gather table IS the input tensor - no on-device table build),
    per-core edge routing, packed gather indices, dstloc one-hot keys, and
    a partition-replicated dinv row for the dst-side scaling.
  - Math: out = relu(dinv_dst * ((sum_e xf[src_e] + xf[dst]) @ W1^T) + b1)
    @ W2^T + b2, where xf = dinv * x.  The W1 transform commutes with the
    aggregation so each core only transforms its own 6272 aggregated rows.
  - Device: dma_gather (1024 idxs/call: single_packet caps at 64 descs/lane;
    the SWDGE ring caps at ~128 slots/lane) fetches per-edge rows from xf,
    one-hot scatter matrices (is_equal against iota) route each edge to its
    dst column, TensorE accumulates agg^T = M^T S in PSUM.  The Pool (Q7)
    engine's descriptor generation (~8.5 ns/edge, engine-serialized) is the
    bottleneck; everything else hides under it.
"""

import sys

import numpy as np

for _p in ("/opt/trn_rl_repo", "/opt/pypackages"):
    if _p not in sys.path:
        sys.path.append(_p)

import concourse.bass as bass
import concourse.tile as tile
from concourse import bacc, mybir
from concourse.tile_rust import add_dep_helper
from concourse.bass_utils import run_bass_kernel_spmd

N = 50000
D = 128
HID = 128
ODIM = 8
CORES = 8
TILE = 128
TPC = 49                      # tiles per core (core 7: 48 real + 1 dummy)
NPC = TPC * TILE              # 6272 nodes per core
NP = CORES * NPC              # 50176 padded node count
LO = 32768                    # int16-safe gather-table split
BATCH_BWS = [512] * 12 + [128]   # 12*512 + 128 = 6272
NBATCH = len(BATCH_BWS)
PAD_DL = 999.0                # dstlocal sentinel: matches no iota column
import os as _os
# chunks (x128 idxs) per dma_gather call: single_packet=True caps a call at
# 64 descs/lane (1024 idxs); with single_packet=False the SWDGE ring cap of
# ~128 slots/lane (~2032 idxs incl sem desc) binds -> 15*128 = 1920 idxs.
GCHUNKS = int(_os.environ.get("GCN_GCHUNKS", "15"))
SINGLE_PACKET = bool(int(_os.environ.get("GCN_SP", "0")))
SGROUP = 8                    # S matrices built per tensor_tensor op

F16 = np.float16
F32 = np.float32


def _preprocess(edge_index):
    """Route/sort real edges; build uniform compile-time meta + arrays."""
    src = np.asarray(edge_index[0], dtype=np.int64)
    dst = np.asarray(edge_index[1], dtype=np.int64)
    order = np.argsort(dst, kind="stable")
    ssrc = src[order].astype(np.int32)
    sdst = dst[order].astype(np.int32)

    counts = np.bincount(dst, minlength=NP)
    deg = counts.astype(np.float64) + 1.0          # self-loop included
    dinv = (1.0 / np.sqrt(deg)).astype(F32)        # pad nodes -> 1.0
    ptr = np.zeros(NP + 1, dtype=np.int64)
    ptr[1:] = np.cumsum(counts)

    # per (core, batch, side): src-index list + batch-relative dst list
    per = [[None] * (2 * NBATCH) for _ in range(CORES)]
    for c in range(CORES):
        for b in range(NBATCH):
            base = c * NPC + b * 512
            bw = BATCH_BWS[b]
            lo_e = ptr[base]
            hi_e = ptr[min(base + bw, NP)]
            s = ssrc[lo_e:hi_e]
            dl = (sdst[lo_e:hi_e] - base).astype(np.int32)
            m = s < LO
            per[c][2 * b] = (s[m], dl[m])
            per[c][2 * b + 1] = (s[~m] - LO, dl[~m])

    # uniform chunk counts + mm-entry lists
    meta = {"batches": []}
    nidx16_tot = 0
    nmm_tot = 0         # matmul entries == dstloc columns
    for b in range(NBATCH):
        bw = BATCH_BWS[b]
        ent = {"bw": bw, "sides": []}
        for side in range(2):
            cmax = max(len(per[c][2 * b + side][0]) for c in range(CORES))
            cmax = max(cmax, 1)
            k = (cmax + TILE - 1) // TILE
            ent["sides"].append(
                {"cmax": cmax, "k": k, "idx_off16": nidx16_tot}
            )
            nidx16_tot += (k * TILE) // 16

        # union dst span per chunk across cores -> per-tile mm entries
        mm = []  # (buf_ci, tile_j, dstloc_col)
        klo = ent["sides"][0]["k"]
        for side in range(2):
            sd = ent["sides"][side]
            k = sd["k"]
            lo_span = np.full(k, np.inf)
            hi_span = np.full(k, -np.inf)
            for c in range(CORES):
                _, dl_l = per[c][2 * b + side]
                n = len(dl_l)
                if n == 0:
                    continue
                nk = (n + TILE - 1) // TILE
                starts = np.arange(nk) * TILE
                mn = np.minimum.reduceat(dl_l, starts)
                mx = np.maximum.reduceat(dl_l, starts)
                lo_span[:nk] = np.minimum(lo_span[:nk], mn)
                hi_span[:nk] = np.maximum(hi_span[:nk], mx)
            for ci in range(k):
                if not np.isfinite(lo_span[ci]):
                    continue   # all-pad chunk on every core: no matmul
                j0 = int(lo_span[ci]) // TILE
                j1 = int(hi_span[ci]) // TILE
                buf_ci = ci if side == 0 else klo + ci
                for j in range(j0, j1 + 1):
                    mm.append((buf_ci, j, nmm_tot))
                    nmm_tot += 1
        ent["mm"] = mm
        meta["batches"].append(ent)
    meta["nidx16"] = nidx16_tot
    meta["nmm"] = nmm_tot
    meta["maxch"] = max(
        e["sides"][0]["k"] + e["sides"][1]["k"] for e in meta["batches"]
    )

    # per-core packed arrays
    dstloc = np.full((CORES, 128, nmm_tot), PAD_DL, dtype=F16)
    srcidx = np.full((CORES, 128, nidx16_tot), 0, dtype=np.int16)
    cnts = [[] for _ in range(CORES)]   # per-call true idx counts, call order
    for c in range(CORES):
        for b in range(NBATCH):
            ent = meta["batches"][b]
            dls = []
            for side in range(2):
                sd = ent["sides"][side]
                idx_l, dl_l = per[c][2 * b + side]
                n = len(idx_l)
                k = sd["k"]
                tot = k * TILE
                # 0 pads gather row 0 harmlessly (PAD_DL zeroes the S rows).
                # (-1 value-trim pads + runtime num_idxs_reg counts were
                # tried: the per-batch buffer memsets they require stalled
                # DVE/Pool and net-regressed ~50us.)
                idx = np.zeros(tot, dtype=np.int16)
                idx[:n] = idx_l.astype(np.int16)
                for p0 in range(0, k, GCHUNKS):
                    pk = min(GCHUNKS, k - p0)
                    cc = min(max(n - p0 * TILE, 0), pk * TILE)
                    if cc == 0:
                        idx[p0 * TILE] = 0   # keep >=1 idx per call
                        cc = 1
                    cnts[c].append(cc)
                srcidx[c][:, sd["idx_off16"] : sd["idx_off16"] + tot // 16] = (
                    np.tile(idx.reshape(tot // 16, 16).T, (8, 1))
                )
                dl = np.full(tot, PAD_DL, dtype=F32)
                dl[:n] = dl_l.astype(F32)
                dls.append(dl.reshape(k, TILE))
            dl_all = np.concatenate(dls, axis=0)  # [klo+khi, 128]
            for buf_ci, j, col in ent["mm"]:
                dstloc[c][:, col] = (dl_all[buf_ci] - j * TILE).astype(F16)
    # entries outside [0,128) (other tile's edges / pads) match no iota col
    dstloc[np.logical_or(dstloc < 0, dstloc >= TILE)] = PAD_DL

    ncalls = len(cnts[0])
    meta["ncalls"] = ncalls
    ncp = (ncalls + 15) // 16 * 16
    cnts_arr = np.zeros((CORES, 1, ncp), dtype=np.int32)
    for c in range(CORES):
        assert len(cnts[c]) == ncalls
        cnts_arr[c, 0, :ncalls] = cnts[c]

    return meta, dinv, dstloc, srcidx, cnts_arr


def _bc_mid(ap2d, g):
    """[128, W] AP -> [128, g, W] with a step-0 middle dim."""
    return bass.AP(ap2d.tensor, ap2d.offset, [ap2d.ap[0], [0, g], ap2d.ap[1]])


def _build_program(meta):
    nc = bacc.Bacc("TRN2", target_bir_lowering=False, debug=False,
                   num_devices=CORES, num_swdge_queues=1)
    dt = mybir.dt

    xf = nc.dram_tensor("xf", [NP, D], dt.float16, kind="ExternalInput")
    xownt_d = nc.dram_tensor("xownt", [128, NPC], dt.float16,
                             kind="ExternalInput")
    dstloc_d = nc.dram_tensor("dstloc", [128, meta["nmm"]], dt.float16,
                              kind="ExternalInput")
    srcidx_d = nc.dram_tensor("srcidx", [128, meta["nidx16"]], dt.int16,
                              kind="ExternalInput")
    iota_d = nc.dram_tensor("iota", [128, 128], dt.float16, kind="ExternalInput")
    dinvrow_d = nc.dram_tensor("dinvrow", [128, NPC], dt.float16,
                               kind="ExternalInput")
    w1t_d = nc.dram_tensor("w1t", [D, HID], dt.float32, kind="ExternalInput")
    b1_d = nc.dram_tensor("b1c", [HID, 1], dt.float32, kind="ExternalInput")
    w2t_d = nc.dram_tensor("w2t", [HID, ODIM], dt.float16, kind="ExternalInput")
    b2_d = nc.dram_tensor("b2c", [ODIM, 1], dt.float32, kind="ExternalInput")
    out_d = nc.dram_tensor("out", [ODIM, NPC], dt.float32, kind="ExternalOutput")

    with tile.TileContext(nc) as tc:
        with (
            tc.tile_pool(name="const", bufs=1) as cpool,
            tc.tile_pool(name="msg", bufs=6) as msg_pool,
            tc.tile_pool(name="sloop", bufs=NBATCH) as sl_pool,
            tc.tile_pool(name="smat", bufs=6) as s_pool,
            tc.tile_pool(name="eptmp", bufs=3) as ep_pool,
            tc.tile_pool(name="psA", bufs=2, space="PSUM") as psA,
            tc.tile_pool(name="psZ", bufs=2, space="PSUM") as psZ,
            tc.tile_pool(name="psO", bufs=2, space="PSUM") as psO,
        ):
            # ---- constants in (srcidx first: gathers depend only on it;
            # per-batch slices so batch b's gathers wait only for slice b) ----
            idx_t = cpool.tile([128, meta["nidx16"]], dt.int16, tag="srcidx")
            for _b in range(NBATCH):
                _s0 = meta["batches"][_b]["sides"][0]["idx_off16"]
                _sd1 = meta["batches"][_b]["sides"][1]
                _s1 = _sd1["idx_off16"] + (_sd1["k"] * TILE) // 16
                nc.sync.dma_start(
                    idx_t[:, _s0:_s1], srcidx_d.ap()[:, _s0:_s1]
                )
            iota_t = cpool.tile([128, 128], dt.float16, tag="iota")
            nc.sync.dma_start(iota_t[:], iota_d.ap())
            dstloc_t = cpool.tile([128, meta["nmm"]], dt.float16, tag="dstloc")
            nc.sync.dma_start(dstloc_t[:], dstloc_d.ap())
            w1t_t = cpool.tile([D, HID], dt.float32, tag="w1t")
            nc.sync.dma_start(w1t_t[:], w1t_d.ap())
            b1_t = cpool.tile([HID, 1], dt.float32, tag="b1")
            nc.sync.dma_start(b1_t[:], b1_d.ap())
            w2t_t = cpool.tile([HID, ODIM], dt.float16, tag="w2t")
            nc.sync.dma_start(w2t_t[:], w2t_d.ap())
            b2_t = cpool.tile([ODIM, 1], dt.float32, tag="b2")
            nc.sync.dma_start(b2_t[:], b2_d.ap())
            dinvr_t = cpool.tile([128, NPC], dt.float16, tag="dinvrow")
            nc.sync.dma_start(dinvr_t[:], dinvrow_d.ap())
            zeros_t = cpool.tile([1, 512], dt.float16, tag="zeros")
            nc.vector.memset(zeros_t[:], 0.0)

            # ---- gathers + one-hot scatter matmuls + epilogue ----
            out_acc = cpool.tile([ODIM, NPC], dt.float32, tag="outacc")
            xs_lo_ap = xf.ap()[0:LO, :]
            xs_hi_ap = xf.ap()[LO:NP, :]

            call_idx = 0

            for b in range(NBATCH):
                ent = meta["batches"][b]
                bw = ent["bw"]
                klo = ent["sides"][0]["k"]
                buf = msg_pool.tile([128, meta["maxch"], D], dt.float16,
                                    tag="msg")
                for side, c0 in ((0, 0), (1, klo)):
                    sd = ent["sides"][side]
                    k = sd["k"]
                    for p0 in range(0, k, GCHUNKS):
                        pk = min(GCHUNKS, k - p0)
                        off = sd["idx_off16"] + (p0 * TILE) // 16
                        nc.gpsimd.dma_gather(
                            out_ap=buf[:, c0 + p0 : c0 + p0 + pk, :],
                            in_ap=xs_lo_ap if side == 0 else xs_hi_ap,
                            idxs_ap=idx_t[:, off : off + (pk * TILE) // 16],
                            num_idxs=pk * TILE,
                            num_idxs_reg=pk * TILE,
                            elem_size=D,
                            single_packet=SINGLE_PACKET,
                            queue_num=0,
                        )
                        call_idx += 1

                # self-loop rows: own batch columns of host-transposed xf
                slt = sl_pool.tile([128, 512], dt.float16, tag="sloop")
                nc.sync.dma_start(
                    slt[:, :bw],
                    xownt_d.ap()[:, b * 512 : b * 512 + bw],
                )

                agg_ps = psA.tile([128, 512], dt.float32, tag="agg")
                nc.tensor.matmul(
                    out=agg_ps[:], lhsT=zeros_t[:, :128], rhs=zeros_t[:],
                    start=True, stop=False, skip_group_check=True,
                )
                mm = ent["mm"]
                nmm = len(mm)
                for g0 in range(0, nmm, SGROUP):
                    gn = min(SGROUP, nmm - g0)
                    col0 = mm[g0][2]
                    s_t = s_pool.tile([128, SGROUP, TILE], dt.float16,
                                      tag="smat")
                    nc.vector.tensor_tensor(
                        out=s_t[:, :gn, :],
                        in0=_bc_mid(iota_t[:], gn),
                        in1=dstloc_t[:, col0 : col0 + gn].to_broadcast(
                            [128, gn, TILE]
                        ),
                        op=mybir.AluOpType.is_equal,
                    )
                    for gi in range(gn):
                        buf_ci, j, _ = mm[g0 + gi]
                        nc.tensor.matmul(
                            out=agg_ps[:, j * TILE : (j + 1) * TILE],
                            lhsT=buf[:, buf_ci, :],
                            rhs=s_t[:, gi, :],
                            start=False,
                            stop=(g0 + gi == nmm - 1),
                            skip_group_check=True,
                        )

                # epilogue for this batch: agg = psum + self-loop rows
                agg_sb = ep_pool.tile([128, 512], dt.float32, tag="aggsb")
                nc.vector.tensor_tensor(
                    out=agg_sb[:, :bw],
                    in0=agg_ps[:, :bw],
                    in1=slt[:, :bw],
                    op=mybir.AluOpType.add,
                )
                z_ps = psZ.tile([128, 512], dt.float32, tag="z")
                nc.tensor.matmul(out=z_ps[:, :bw], lhsT=w1t_t[:],
                                 rhs=agg_sb[:, :bw], start=True, stop=True)
                z2_sb = ep_pool.tile([128, 512], dt.float32, tag="z2")
                nc.vector.tensor_tensor(
                    out=z2_sb[:, :bw],
                    in0=z_ps[:, :bw],
                    in1=dinvr_t[:, b * 512 : b * 512 + bw],
                    op=mybir.AluOpType.mult,
                )
                h_sb = ep_pool.tile([128, 512], dt.float16, tag="h")
                nc.scalar.activation(h_sb[:, :bw], z2_sb[:, :bw],
                                     mybir.ActivationFunctionType.Relu,
                                     bias=b1_t[:])
                o_ps = psO.tile([ODIM, 512], dt.float32, tag="o")
                nc.tensor.matmul(out=o_ps[:, :bw], lhsT=w2t_t[:],
                                 rhs=h_sb[:, :bw], start=True, stop=True)
                nc.vector.tensor_scalar(
                    out=out_acc[:, b * 512 : b * 512 + bw],
                    in0=o_ps[:, :bw],
                    scalar1=b2_t[:],
                    scalar2=None,
                    op0=mybir.AluOpType.add,
                )

            nc.sync.dma_start(out_d.ap(), out_acc[:])

    nc.compile()
    return nc


_CACHE = {}
last_results = None


def kernel(x, edge_index, W1, b1, W2, b2):
    import os

    meta, dinv, dstloc, srcidx, cnts_arr = _preprocess(edge_index)

    xf = np.zeros((NP, D), dtype=F16)
    xf[:N] = (np.asarray(x, dtype=F32) * dinv[:N, None]).astype(F16)
    iota = np.broadcast_to(np.arange(128, dtype=F16), (128, 128)).copy()
    w1t = np.asarray(W1, dtype=F32).T.copy()              # [D, HID]
    b1c = np.asarray(b1, dtype=F32).reshape(HID, 1)
    w2t = np.asarray(W2, dtype=F32).T.astype(F16).copy()  # [HID, ODIM]
    b2c = np.asarray(b2, dtype=F32).reshape(ODIM, 1)

    key = tuple(
        (e["bw"], tuple(e["mm"]))
        + tuple((sd["cmax"], sd["k"]) for sd in e["sides"])
        for e in meta["batches"]
    )
    if key not in _CACHE:
        _CACHE[key] = _build_program(meta)
    nc = _CACHE[key]

    in_maps = []
    for c in range(CORES):
        sl = slice(c * NPC, (c + 1) * NPC)
        dinvrow = np.broadcast_to(
            dinv[sl].astype(F16), (128, NPC)
        ).copy()
        in_maps.append(
            {
                "xf": xf,
                "xownt": np.ascontiguousarray(xf[sl].T),
                "dstloc": dstloc[c],
                "srcidx": srcidx[c],
                "iota": iota,
                "dinvrow": dinvrow,
                "w1t": w1t,
                "b1c": b1c,
                "w2t": w2t,
                "b2c": b2c,
            }
        )

    trace = bool(os.environ.get("GCN_TRACE"))
    res = run_bass_kernel_spmd(
        nc, in_maps, core_ids=list(range(CORES)), trace=trace
    )
    global last_results
    last_results = res
    big = np.concatenate([res.results[c]["out"] for c in range(CORES)], axis=1)
    return np.ascontiguousarray(big[:, :N].T).astype(F32)
